# revision 60
# baseline (speedup 1.0000x reference)
"""HeteroGNN (GCN over authors + SAGE over papers) on 8 Trainium2 NeuronCores.

Strategy (graph/data parallel, per the sharding hint):
  - Papers and authors are sharded across the 8 cores by destination node.
  - Host preprocessing sorts each edge set by destination tile and emits, per
    core: an int16 gather-index table (lo/hi split for the 32k int16 range)
    plus compact per-edge (dst_rel, weight) tables.  On device, a message
    pass for a group of dst tiles is:
        messages = dma_gather(table, idx)              # one big SWDGE call
        ind[e,d] = (iota[d] == dst_rel[e]) * w[e]      # DVE build, no HBM
        psum[f,d] += msg_chunk.T @ ind_chunk           # PE segment-sum
  - GCN layer-0 output a1 is exchanged with one AllGather; SAGE layer-1
    gathers a1 rows from the shared table.
  - The pooler runs in 512-paper slabs and its contribution to SAGE layer 0
    (xp @ Wr0 + b) is pre-accumulated into an SBUF "static" table; likewise
    p1 @ Wr1 + b1 is folded into a static table for SAGE layer 1.
  - The unused second GCN layer (a2) of the reference is dead code, skipped.
"""

import sys

sys.path.insert(0, "/opt/trn_rl_repo")

import numpy as np
import ml_dtypes

BF16 = ml_dtypes.bfloat16

# ---- problem constants (from spec) ----
N_CORES = 8
N_AUTHOR = 50000
N_PAPER = 25000
D = 128          # author feature dim == hidden dim
D_BERT = 768
FEAT = 12
OUT_D = 64
VOCAB = 31090

A_PC = N_AUTHOR // N_CORES      # 6250 authors per core
A_TILES = 49                    # ceil(6250/128)
A_PC_PAD = A_TILES * 128        # 6272
N_A_PAD = N_CORES * A_PC_PAD    # 50176 padded author table rows
P_PC = N_PAPER // N_CORES       # 3125 papers per core
P_TILES = 25
P_PC_PAD = P_TILES * 128        # 3200
SPLIT = 32768                   # int16 index limit for dma_gather
GRP_C = 5                       # GCN: 10 groups x 5 tiles
GRP_W = 2                       # SAGE: groups of 2 tiles (512B rows)

TRACE = False
LAST = {}

_CACHE = {}


def _pad_author_ids(v):
    """Map real author id -> padded table row (core-concat layout)."""
    return (v // A_PC) * A_PC_PAD + (v % A_PC)


def _wrap_idx(flat):
    """Lay out gather indices as [128, n/16] int16 (16-wrap, tiled x8)."""
    n = flat.shape[0]
    assert n % 16 == 0
    w = flat.reshape(n // 16, 16).T.astype(np.int16)  # [16, n/16]
    return np.tile(w, (8, 1))                          # [128, n/16]


def _prep_graph(src_pad, dst, w, n_dst_pc, n_tiles, grp,
                self_w=None):
    """Partition edges by destination shard/tile, split lo/hi by src row,
    group tiles for batched gathers, and build per-core idx + (dst_rel, w)
    tables.  Chunk counts are per-tile (shared across cores for SPMD).

    Returns (cfg, [(idx [128, TOT/16] i16, drel [128, TOTC] bf16,
                    w [128, TOTC] bf16) per core]).
    """
    E = src_pad.shape[0]
    core = dst // n_dst_pc
    ld = dst - core * n_dst_pc
    tile = ld >> 7
    drel = ld & 127
    hi = (src_pad >= SPLIT).astype(np.int64)

    key = (core * n_tiles + tile) * 2 + hi
    order = np.argsort(key, kind="stable")
    key_s = key[order]
    counts = np.bincount(key, minlength=N_CORES * n_tiles * 2)
    starts = np.concatenate([[0], np.cumsum(counts)[:-1]])
    rank = np.arange(E, dtype=np.int64) - starts[key_s]

    n_lo = counts[0::2].reshape(N_CORES, n_tiles)
    n_hi = counts[1::2].reshape(N_CORES, n_tiles)
    CLO = np.maximum(1, -(-n_lo.max(axis=0) // 128))  # [n_tiles]
    CHI = np.maximum(1, -(-n_hi.max(axis=0) // 128))
    CT = CLO + CHI

    gof = list(range(0, n_tiles, grp))
    gsz = [min(grp, n_tiles - g0) for g0 in gof]
    glo = [int(CLO[g0:g0 + s].sum()) for g0, s in zip(gof, gsz)]
    ghi = [int(CHI[g0:g0 + s].sum()) for g0, s in zip(gof, gsz)]
    gchunks = [a + b for a, b in zip(glo, ghi)]
    gbase = np.concatenate([[0], np.cumsum(np.array(gchunks) * 128)[:-1]])
    TOT = int(sum(gchunks)) * 128

    lo_off = np.zeros(n_tiles, np.int64)
    hi_off = np.zeros(n_tiles, np.int64)
    lo_cbase = np.zeros(n_tiles, np.int64)
    hi_cbase = np.zeros(n_tiles, np.int64)
    for gi, g0 in enumerate(gof):
        s = gsz[gi]
        cl = np.concatenate([[0], np.cumsum(CLO[g0:g0 + s])[:-1]])
        ch = np.concatenate([[0], np.cumsum(CHI[g0:g0 + s])[:-1]])
        for j in range(s):
            t = g0 + j
            lo_cbase[t] = cl[j]
            hi_cbase[t] = glo[gi] + ch[j]
            lo_off[t] = gbase[gi] + cl[j] * 128
            hi_off[t] = gbase[gi] + (glo[gi] + ch[j]) * 128

    if self_w is not None:
        CT = CT + 1  # trailing per-tile self chunk, filled by plain DMA
    chunk_off = np.concatenate([[0], np.cumsum(CT)[:-1]])
    TOTC = int(CT.sum())

    src_s = src_pad[order]
    drel_s = drel[order]
    w_s = w[order]
    core_s = core[order]
    tile_s = tile[order]
    hi_s = hi[order]
    idx_val = (src_s - hi_s * SPLIT).astype(np.int16)
    c_in_tile = np.where(hi_s == 0, rank >> 7, CLO[tile_s] + (rank >> 7))
    pos = np.where(hi_s == 0, lo_off[tile_s] + rank, hi_off[tile_s] + rank)
    colc = chunk_off[tile_s] + c_in_tile
    ep = rank & 127

    per_core = []
    for c in range(N_CORES):
        m = core_s == c
        flat = np.zeros(TOT, dtype=np.int16)
        flat[pos[m]] = idx_val[m]
        drel_t = np.zeros((128, TOTC), dtype=BF16)
        w_t = np.zeros((128, TOTC), dtype=BF16)
        drel_t[ep[m], colc[m]] = drel_s[m].astype(BF16)
        w_t[ep[m], colc[m]] = w_s[m].astype(BF16)
        if self_w is not None:
            selfcols = chunk_off + (CT - 1)  # [n_tiles]
            drel_t[:, selfcols] = np.arange(128, dtype=np.float32)[:, None]
            w_t[:, selfcols] = self_w[c].T.astype(BF16)
        per_core.append((_wrap_idx(flat), drel_t, w_t))

    cfg = dict(
        CLO=[int(x) for x in CLO], CHI=[int(x) for x in CHI],
        CT=[int(x) for x in CT],
        gof=gof, gsz=gsz, glo=glo, ghi=ghi, gchunks=gchunks,
        gbase=[int(x) for x in gbase],
        lo_cbase=[int(x) for x in lo_cbase],
        hi_cbase=[int(x) for x in hi_cbase],
        chunk_off=[int(x) for x in chunk_off],
        TOT=TOT, TOTC=TOTC, n_tiles=n_tiles,
        self_chunk=self_w is not None,
    )
    return cfg, per_core


def _prep(inputs):
    """Host preprocessing: per-core in_maps + static builder config."""
    f32 = np.float32
    x_author = np.asarray(inputs["x_author"], f32)
    paper_tokens = np.asarray(inputs["paper_tokens"])
    paper_feat = np.asarray(inputs["paper_feat"], f32)
    edge_collab = np.asarray(inputs["edge_collab"], np.int64)
    writes_src = np.asarray(inputs["writes_src"], np.int64)
    writes_dst = np.asarray(inputs["writes_dst"], np.int64)

    # -- padded bf16 author table, pre-interleaved as the low half of the
    # [xa | a1] SAGE message table (the device fills the a1 half after the
    # AllGather; idempotent across NEFF re-runs).
    xaa1 = np.zeros((N_A_PAD, 2 * D), dtype=BF16)
    rows = _pad_author_ids(np.arange(N_AUTHOR))
    xaa1[rows, :D] = x_author.astype(BF16)
    xa_pad = xaa1[:, :D]

    emb = np.asarray(inputs["embed_table"], f32).astype(BF16)

    # -- GCN (collab -> authors): norm = dinv[src]*dinv[dst]; self loop dinv^2
    src_c, dst_c = edge_collab[0], edge_collab[1]
    deg = np.bincount(dst_c, minlength=N_AUTHOR).astype(f32) + 1.0
    dinv = 1.0 / np.sqrt(deg)
    # self-loop messages come from a plain DMA of the core's own shard;
    # only their dinv^2 weights go through the (drel, w) tables
    sw = np.zeros((N_CORES, A_TILES, 128), f32)
    d2 = (dinv * dinv).reshape(N_CORES, A_PC)
    sw.reshape(N_CORES, A_TILES * 128)[:, :A_PC] = d2
    cfg_c, pc_c = _prep_graph(
        _pad_author_ids(src_c), dst_c,
        (dinv[src_c] * dinv[dst_c]).astype(f32), A_PC, A_TILES, grp=GRP_C,
        self_w=sw)

    # -- SAGE (writes -> papers): weight 1/max(cnt,1)
    cnt = np.bincount(writes_dst, minlength=N_PAPER).astype(f32)
    s_w = (1.0 / np.maximum(cnt, 1.0))[writes_dst].astype(f32)
    cfg_w, pc_w = _prep_graph(
        _pad_author_ids(writes_src), writes_dst, s_w, P_PC, P_TILES, grp=GRP_W)

    # -- weights (host-reshaped to device layouts, bf16)
    def brow(name, n):
        return np.asarray(inputs[name], f32).reshape(1, n).astype(BF16)

    Wp = np.asarray(inputs["Wp"], f32)  # [768, 768] [k, f]
    wp_sb = Wp.reshape(6, 128, D_BERT).transpose(1, 0, 2).reshape(128, 6 * D_BERT)
    Wr0 = np.asarray(inputs["sage_Wr0"], f32)  # [780, 128]
    wr0_sb = Wr0[:768].reshape(6, 128, 128).transpose(1, 0, 2).reshape(128, 6 * 128)
    wr0f = Wr0[768:780]  # [12, 128]
    bp_col = np.asarray(inputs["bp"], f32).reshape(6, 128).T  # [128, 6]

    iota = np.tile(np.arange(128, dtype=f32), (128, 1))  # [128, 128]
    ident = np.eye(128, dtype=f32)

    consts = dict(
        wp=wp_sb.astype(BF16),
        bpcol=bp_col.astype(BF16),
        w0=np.asarray(inputs["gcn_W0"], f32).astype(BF16),
        b0=brow("gcn_b0", D),
        wl0=np.asarray(inputs["sage_Wl0"], f32).astype(BF16),
        wr0=wr0_sb.astype(BF16),
        wr0f=wr0f.astype(BF16),
        sb0col=np.asarray(inputs["sage_b0"], f32).reshape(128, 1).astype(BF16),
        wl1=np.asarray(inputs["sage_Wl1"], f32).astype(BF16),
        wr1=np.asarray(inputs["sage_Wr1"], f32).astype(BF16),
        sb1col=np.asarray(inputs["sage_b1"], f32).reshape(128, 1).astype(BF16),
        linw=np.asarray(inputs["lin_W"], f32).astype(BF16),
        linb=brow("lin_b", OUT_D),
        ones=np.ones((1, 128), dtype=BF16),
        iota=iota.astype(BF16),
        ident=ident.astype(BF16),
    )

    in_maps = []
    for c in range(N_CORES):
        p0, p1 = c * P_PC, (c + 1) * P_PC
        cls = np.zeros(P_PC_PAD, dtype=np.int64)
        cls[:P_PC] = paper_tokens[p0:p1, 0]
        featT = np.zeros((FEAT, P_PC_PAD), dtype=BF16)
        featT[:, :P_PC] = paper_feat[p0:p1].T.astype(BF16)
        idx_c, drel_c, w_c = pc_c[c]
        idx_w, drel_w, w_w = pc_w[c]
        m = dict(
            emb=emb,
            clsidx=_wrap_idx(cls),
            featT=featT,
            xaa1=xaa1,
            xa_my=np.ascontiguousarray(
                xa_pad[c * A_PC_PAD:(c + 1) * A_PC_PAD]),
            idxc=idx_c, drelc=drel_c, wc=w_c,
            idxw=idx_w, drelw=drel_w, ww=w_w,
            **consts,
        )
        in_maps.append(m)

    return in_maps, (cfg_c, cfg_w)


def _cfg_key(cfg):
    return (tuple(cfg["CLO"]), tuple(cfg["CHI"]))


def _build(cfg_c, cfg_w):
    """Build the SPMD Bass program (shared by all 8 cores)."""
    import concourse.bacc as bacc
    import concourse.mybir as mybir
    from concourse.tile import TileContext

    dt = mybir.dt
    AF = mybir.ActivationFunctionType
    ALU = mybir.AluOpType

    nc = bacc.Bacc("TRN2", target_bir_lowering=False, debug=False,
                   num_devices=N_CORES, num_swdge_queues=4)

    # Round-robin over the 4 SWDGE queues so the SDMA drain of call N
    # overlaps the Q7 descriptor-gen of call N+1.  Tile tracks SWDGE
    # completion on 8 lanes assigned in execution order, assuming each
    # lane's DMAs finish FIFO; cross-queue completions reorder, so we (a)
    # chain all gathers with no-sync deps to pin execution order to
    # emission order and (b) rotate queues mod 4 — every lane (mod 8)
    # then maps to a fixed queue (mod 4), keeping each lane FIFO.
    qstate = {"q": 0, "hist": [], "barrier": False}

    def gather_rr(dst3, table, idx_ap, n, elem, single_packet=False,
                  transpose=False, elem_step=None):
        from concourse.instruction_name_ordered_set import (
            InstructionNameOrderedSet)
        q = qstate["q"]
        qstate["q"] = (q + 1) % 4
        bi = nc.gpsimd.dma_gather(dst3, table, idx_ap, n, n, elem,
                                  elem_step=elem_step,
                                  single_packet=single_packet,
                                  transpose=transpose, queue_num=q)
        hist = qstate["hist"]
        if hist:
            deps = InstructionNameOrderedSet()
            deps.add(hist[-1])
            bi.ins.add_nosync_dependencies_from(deps)
        if qstate["barrier"] and hist:
            # Phase boundary (collectives insert queue drains): wait for
            # ALL in-flight gathers before issuing the next one.
            qstate["barrier"] = False
            sdeps = InstructionNameOrderedSet()
            for nm in hist[-8:]:
                sdeps.add(nm)
            bi.ins.add_sync_dependencies_from(sdeps)
        elif len(hist) >= 8:
            # ≤1 DMA in flight per Tile DMASW lane (8 lanes): wait for the
            # lane predecessor's completion before issuing, so the shared
            # lane semaphore's 16 per-engine increments never interleave.
            sdeps = InstructionNameOrderedSet()
            sdeps.add(hist[-8])
            bi.ins.add_sync_dependencies_from(sdeps)
        hist.append(bi.ins.name)

    # ---- I/O declarations
    def din(name, shape, dtype=dt.bfloat16):
        return nc.dram_tensor(name, list(shape), dtype, kind="ExternalInput").ap()

    emb = din("emb", (VOCAB, D_BERT))
    clsidx = din("clsidx", (128, P_PC_PAD // 16), dt.int16)
    featT = din("featT", (FEAT, P_PC_PAD))
    xaa1 = din("xaa1", (N_A_PAD, 2 * D))
    xa_my = din("xa_my", (A_PC_PAD, D))
    wp = din("wp", (128, 6 * D_BERT))
    bpcol = din("bpcol", (128, 6))
    w0 = din("w0", (D, D))
    b0 = din("b0", (1, D))
    wl0 = din("wl0", (D, D))
    wr0 = din("wr0", (128, 6 * 128))
    wr0f = din("wr0f", (FEAT, D))
    sb0col = din("sb0col", (128, 1))
    wl1 = din("wl1", (D, D))
    wr1 = din("wr1", (D, D))
    sb1col = din("sb1col", (128, 1))
    linw = din("linw", (D, OUT_D))
    linb = din("linb", (1, OUT_D))
    ones = din("ones", (1, 128))
    iota = din("iota", (128, 128))
    ident = din("ident", (128, 128))
    idxc = din("idxc", (128, cfg_c["TOT"] // 16), dt.int16)
    drelc = din("drelc", (128, cfg_c["TOTC"]))
    wc = din("wc", (128, cfg_c["TOTC"]))
    idxw = din("idxw", (128, cfg_w["TOT"] // 16), dt.int16)
    drelw = din("drelw", (128, cfg_w["TOTC"]))
    ww = din("ww", (128, cfg_w["TOTC"]))
    out = nc.dram_tensor("out", [P_PC_PAD, OUT_D], dt.float32,
                         kind="ExternalOutput").ap()

    MSGW = max((max(cfg_c["gchunks"]) + GRP_C) * 128,
           max(cfg_w["gchunks"]) * 256)
    INDW = max(max(cfg_c["CT"]), max(cfg_w["CT"])) * 128

    with TileContext(nc) as tc:
        with (
            tc.tile_pool(name="const", bufs=1) as constp,
            tc.tile_pool(name="big", bufs=1) as bigp,
            tc.tile_pool(name="msg", bufs=3) as msgp,
            tc.tile_pool(name="ind", bufs=3) as indp,
            tc.tile_pool(name="cls", bufs=2) as clsp,
            tc.tile_pool(name="work", bufs=3) as workp,
            tc.tile_pool(name="psum", bufs=1, space="PSUM") as psump,
            tc.tile_pool(name="dram", bufs=1, space="DRAM") as dramp,
        ):
            # ---- constants to SBUF
            def load_const(ap_dram, name):
                t = constp.tile(list(ap_dram.shape), ap_dram.dtype, name=name)
                nc.sync.dma_start(out=t, in_=ap_dram)
                return t

            # GCN-critical tables first: the first gathers and their DVE
            # consumers gate the pipeline fill.
            idxc_sb = load_const(idxc, "idxc_sb")
            drelc_sb = load_const(drelc, "drelc_sb")
            wc_sb = load_const(wc, "wc_sb")
            iota_sb = load_const(iota, "iota_sb")
            w0_sb = load_const(w0, "w0_sb")
            b0_sb = load_const(b0, "b0_sb")
            ones_sb = load_const(ones, "ones_sb")
            idxw_sb = load_const(idxw, "idxw_sb")
            drelw_sb = load_const(drelw, "drelw_sb")
            ww_sb = load_const(ww, "ww_sb")
            wp_sb = load_const(wp, "wp_sb")
            bp_sb = load_const(bpcol, "bp_sb")
            wl0_sb = load_const(wl0, "wl0_sb")
            wr0_sb = load_const(wr0, "wr0_sb")
            wr0f_sb = load_const(wr0f, "wr0f_sb")
            sb0_sb = load_const(sb0col, "sb0_sb")
            wl1_sb = load_const(wl1, "wl1_sb")
            wr1_sb = load_const(wr1, "wr1_sb")
            sb1_sb = load_const(sb1col, "sb1_sb")
            linw_sb = load_const(linw, "linw_sb")
            linb_sb = load_const(linb, "linb_sb")
            ident_sb = load_const(ident, "ident_sb")
            clsidx_sb = load_const(clsidx, "clsidx_sb")
            featT_sb = load_const(featT, "featT_sb")

            s0staticT = bigp.tile([128, P_PC_PAD], dt.bfloat16, name="s0staticT")

            a1_shard = dramp.tile([A_PC_PAD, D], dt.bfloat16, name="a1_shard")
            R1 = 24 * 128  # author rows in the first collective half
            R2 = A_PC_PAD - R1
            a1_h1 = dramp.tile([N_CORES * R1, D], dt.bfloat16,
                               addr_space="Shared", name="a1_h1")
            a1_h2 = dramp.tile([N_CORES * R2, D], dt.bfloat16,
                               addr_space="Shared", name="a1_h2")
            # xa half is pre-interleaved by the host; only the a1 half is
            # written on device (idempotent across NEFF re-runs).
            xa_a1 = xaa1
            xa_a1_3d = xaa1[:, D:2 * D].rearrange("(c r) d -> c r d",
                                                  c=N_CORES)

            iota1 = iota_sb[:, :].rearrange("p (o e) -> p o e", o=1)

            # =========== shared message-pass emitter
            def message_pass(cfg, idx_sb, drel_sb, w_sb, table_lo, table_hi,
                             consume, pname, elem=128, agg_w=128,
                             self_table=None, gsel=None, table_step=None):
                CLO, CT = cfg["CLO"], cfg["CT"]
                nself = 1 if cfg.get("self_chunk") else 0
                for gi in (gsel if gsel is not None
                           else range(len(cfg["gof"]))):
                    g0 = cfg["gof"][gi]
                    s = cfg["gsz"][gi]
                    nch = cfg["gchunks"][gi]
                    glo_g = cfg["glo"][gi]
                    msg = msgp.tile([128, MSGW], dt.bfloat16, tag="msg",
                                    name=f"msg_{pname}")
                    m3 = msg[:, :(nch + nself * s) * elem].rearrange(
                        "p (c e) -> p c e", e=elem)
                    for j in range(s * nself):
                        t = g0 + j
                        nc.sync.dma_start(
                            out=m3[:, nch + j, :],
                            in_=self_table[t * 128:(t + 1) * 128, :])
                    base = cfg["gbase"][gi]
                    Nlo = glo_g * 128
                    Nhi = cfg["ghi"][gi] * 128

                    def gath(dst3, table, col0, nidx):
                        done = 0
                        while done < nidx:
                            n = min(8192, nidx - done)
                            gather_rr(
                                dst3[:, done // 128:(done + n) // 128, :],
                                table,
                                idx_sb[:, (col0 + done) // 16:
                                       (col0 + done + n) // 16],
                                n, elem, elem_step=table_step)
                            done += n

                    gath(m3[:, :glo_g, :], table_lo, base, Nlo)
                    gath(m3[:, glo_g:nch, :], table_hi, base + Nlo, Nhi)
                    for j in range(s):
                        t = g0 + j
                        C = CT[t]
                        co = cfg["chunk_off"][t]
                        ind = indp.tile([128, INDW], dt.bfloat16, tag="ind",
                                        name=f"ind_{pname}")
                        i3 = ind[:, :C * 128].rearrange("p (c e) -> p c e",
                                                        e=128)
                        db = drel_sb[:, co:co + C].rearrange(
                            "p (c o) -> p c o", o=1).to_broadcast([128, C, 128])
                        wb = w_sb[:, co:co + C].rearrange(
                            "p (c o) -> p c o", o=1).to_broadcast([128, C, 128])
                        ib = iota1.to_broadcast([128, C, 128])
                        nc.vector.tensor_tensor(i3, ib, db, ALU.is_equal)
                        nc.vector.tensor_tensor(i3, i3, wb, ALU.mult)
                        agg = psump.tile([128, 256], dt.float32, tag="agg",
                                         name=f"agg_{pname}", bufs=2)
                        lob = cfg["lo_cbase"][t]
                        hib = cfg["hi_cbase"][t]
                        for h in range(elem // 128):
                            for k in range(C):
                                if k >= C - nself:
                                    mi = nch + j
                                elif k < CLO[t]:
                                    mi = lob + k
                                else:
                                    mi = hib + (k - CLO[t])
                                nc.tensor.matmul(
                                    agg[:, h * 128:(h + 1) * 128],
                                    lhsT=m3[:, mi, h * 128:(h + 1) * 128],
                                    rhs=ind[:, k * 128:(k + 1) * 128],
                                    start=(k == 0), stop=(k == C - 1),
                                )
                        aggsb = workp.tile([128, 256], dt.bfloat16,
                                           tag="aggsb", name=f"aggsb_{pname}")
                        nc.scalar.activation(aggsb[:, :agg_w],
                                             agg[:, :agg_w], AF.Copy)
                        consume(t, aggsb)

            # =========== GCN layer 0 -> a1 shard, then AllGather
            def gcn_consume(t, aggsb):
                lp = psump.tile([128, 128], dt.float32, tag="layer",
                                name="lp_gcn", bufs=2)
                nc.tensor.matmul(lp, lhsT=aggsb[:, 0:128], rhs=w0_sb,
                                 start=True, stop=False)
                nc.tensor.matmul(lp, lhsT=ones_sb, rhs=b0_sb,
                                 start=False, stop=True)
                a1sb = workp.tile([128, 128], dt.bfloat16, tag="a1sb",
                                  name="a1sb")
                nc.scalar.activation(a1sb, lp, AF.Relu)
                nc.sync.dma_start(out=a1_shard[t * 128:(t + 1) * 128, :],
                                  in_=a1sb)

            with nc.named_scope("gcn"):
                message_pass(cfg_c, idxc_sb, drelc_sb, wc_sb,
                             xaa1[0:SPLIT, 0:D], xaa1[SPLIT:N_A_PAD, 0:D],
                             gcn_consume, "gcn", self_table=xa_my,
                             gsel=[0, 1, 2, 3, 4], table_step=2 * D)
            with nc.named_scope("ag1"):
                nc.gpsimd.collective_compute(
                    "AllGather", ALU.bypass,
                    replica_groups=[list(range(N_CORES))],
                    ins=[a1_shard[0:R1, :]], outs=[a1_h1.opt()],
                )
                nc.sync.dma_start(
                    out=xa_a1_3d[:, 0:R1, :],
                    in_=a1_h1[:, :].rearrange("(c r) d -> c r d", c=N_CORES))
            qstate["barrier"] = True
            with nc.named_scope("gcn2"):
                message_pass(cfg_c, idxc_sb, drelc_sb, wc_sb,
                             xaa1[0:SPLIT, 0:D], xaa1[SPLIT:N_A_PAD, 0:D],
                             gcn_consume, "gcn2", self_table=xa_my,
                             gsel=[5, 6, 7, 8, 9], table_step=2 * D)

            # =========== pooler + SAGE-0 static part, in 512-paper slabs
            qstate["barrier"] = True
            with nc.named_scope("pooler"):
                slabs = []
                off = 0
                while off < P_PC_PAD:
                    wdt = min(512, P_PC_PAD - off)
                    slabs.append((off, wdt))
                    off += wdt
                for soff, W in slabs:
                    clsT = clsp.tile([128, 6 * 512], dt.bfloat16, tag="cls",
                                     name="clsT")
                    c3 = clsT[:, :6 * W].rearrange("p (c e) -> p c e", e=W)
                    gather_rr(c3, emb,
                              clsidx_sb[:, soff // 16:(soff + W) // 16],
                              W, D_BERT, single_packet=True, transpose=True)
                    stat = psump.tile([128, 512], dt.float32, tag="stat",
                                      name="stat", bufs=1)
                    for ft in range(6):
                        ps = psump.tile([128, 512], dt.float32, tag="pool",
                                        name="ps_pool", bufs=2)
                        for cc in range(6):
                            nc.tensor.matmul(
                                ps[:, :W],
                                lhsT=wp_sb[:, cc * D_BERT + ft * 128:
                                           cc * D_BERT + ft * 128 + 128],
                                rhs=c3[:, cc, :],
                                start=(cc == 0), stop=(cc == 5),
                            )
                        pool_sb = workp.tile([128, 512], dt.bfloat16,
                                             tag="poolsb", name="pool_sb")
                        nc.scalar.activation(pool_sb[:, :W], ps[:, :W],
                                             AF.Tanh,
                                             bias=bp_sb[:, ft:ft + 1])
                        nc.tensor.matmul(
                            stat[:, :W],
                            lhsT=wr0_sb[:, ft * 128:(ft + 1) * 128],
                            rhs=pool_sb[:, :W],
                            start=(ft == 0), stop=False)
                    nc.tensor.matmul(stat[:, :W], lhsT=wr0f_sb,
                                     rhs=featT_sb[:, soff:soff + W],
                                     start=False, stop=True)
                    nc.scalar.activation(s0staticT[:, soff:soff + W],
                                         stat[:, :W], AF.Identity,
                                         bias=sb0_sb)

            with nc.named_scope("allgather"):
                nc.gpsimd.collective_compute(
                    "AllGather", ALU.bypass,
                    replica_groups=[list(range(N_CORES))],
                    ins=[a1_shard[R1:A_PC_PAD, :]], outs=[a1_h2.opt()],
                )
                nc.sync.dma_start(
                    out=xa_a1_3d[:, R1:A_PC_PAD, :],
                    in_=a1_h2[:, :].rearrange("(c r) d -> c r d", c=N_CORES))

            # =========== fused SAGE layers 0+1 + head -> out
            def sage_consume(t, aggsb):
                mean0 = aggsb[:, 0:128]
                mean1 = aggsb[:, 128:256]
                pp0 = psump.tile([128, 128], dt.float32, tag="layer",
                                 name="pp0", bufs=2)
                nc.tensor.matmul(pp0, lhsT=wl0_sb, rhs=mean0,
                                 start=True, stop=False)
                nc.tensor.matmul(pp0, lhsT=ident_sb,
                                 rhs=s0staticT[:, t * 128:(t + 1) * 128],
                                 start=False, stop=True)
                p1sb = workp.tile([128, 128], dt.bfloat16, tag="p1sb",
                                  name="p1sb")
                nc.scalar.activation(p1sb, pp0, AF.Relu)
                pp1 = psump.tile([128, 128], dt.float32, tag="layer",
                                 name="pp1", bufs=2)
                nc.tensor.matmul(pp1, lhsT=wl1_sb, rhs=mean1,
                                 start=True, stop=False)
                nc.tensor.matmul(pp1, lhsT=wr1_sb, rhs=p1sb,
                                 start=False, stop=True)
                p2sb = workp.tile([128, 128], dt.bfloat16, tag="p2sb",
                                  name="p2sb")
                nc.scalar.activation(p2sb, pp1, AF.Relu, bias=sb1_sb)
                hp = psump.tile([128, OUT_D], dt.float32, tag="head",
                                name="hp", bufs=1)
                nc.tensor.matmul(hp, lhsT=p2sb, rhs=linw_sb,
                                 start=True, stop=False)
                nc.tensor.matmul(hp, lhsT=ones_sb, rhs=linb_sb,
                                 start=False, stop=True)
                outsb = workp.tile([128, OUT_D], dt.float32, tag="outsb",
                                   name="outsb")
                nc.vector.tensor_copy(outsb, hp)
                nc.sync.dma_start(out=out[t * 128:(t + 1) * 128, :], in_=outsb)

            qstate["barrier"] = True
            with nc.named_scope("sage"):
                message_pass(cfg_w, idxw_sb, drelw_sb, ww_sb,
                             xa_a1[0:SPLIT, :], xa_a1[SPLIT:N_A_PAD, :],
                             sage_consume, "sg", elem=256, agg_w=256)

    nc.compile()
    return nc


def kernel(**inputs):
    from concourse import bass_utils

    in_maps, (cfg_c, cfg_w) = _prep(inputs)

    key = (_cfg_key(cfg_c), _cfg_key(cfg_w))
    if key in _CACHE:
        nc = _CACHE[key]
    else:
        nc = _build(cfg_c, cfg_w)
        _CACHE[key] = nc

    res = bass_utils.run_bass_kernel_spmd(
        nc, in_maps, core_ids=list(range(N_CORES)), trace=TRACE)
    LAST["exec_time_ns"] = res.exec_time_ns
    LAST["results"] = res

    pieces = [res.results[c]["out"][:P_PC] for c in range(N_CORES)]
    return np.concatenate(pieces, axis=0).astype(np.float32)



# revision 62
# speedup vs baseline: 1.0474x; 1.0474x over previous
"""HeteroGNN (GCN over authors + SAGE over papers) on 8 Trainium2 NeuronCores.

Strategy (graph/data parallel, per the sharding hint):
  - Papers and authors are sharded across the 8 cores by destination node.
  - Host preprocessing sorts each edge set by destination tile and emits, per
    core: an int16 gather-index table (lo/hi split for the 32k int16 range)
    plus compact per-edge (dst_rel, weight) tables.  On device, a message
    pass for a group of dst tiles is:
        messages = dma_gather(table, idx)              # one big SWDGE call
        ind[e,d] = (iota[d] == dst_rel[e]) * w[e]      # DVE build, no HBM
        psum[f,d] += msg_chunk.T @ ind_chunk           # PE segment-sum
  - GCN layer-0 output a1 is exchanged with one AllGather; SAGE layer-1
    gathers a1 rows from the shared table.
  - The pooler runs in 512-paper slabs and its contribution to SAGE layer 0
    (xp @ Wr0 + b) is pre-accumulated into an SBUF "static" table; likewise
    p1 @ Wr1 + b1 is folded into a static table for SAGE layer 1.
  - The unused second GCN layer (a2) of the reference is dead code, skipped.
"""

import sys

sys.path.insert(0, "/opt/trn_rl_repo")

import numpy as np
import ml_dtypes

BF16 = ml_dtypes.bfloat16

# ---- problem constants (from spec) ----
N_CORES = 8
N_AUTHOR = 50000
N_PAPER = 25000
D = 128          # author feature dim == hidden dim
D_BERT = 768
FEAT = 12
OUT_D = 64
VOCAB = 31090

A_PC = N_AUTHOR // N_CORES      # 6250 authors per core
A_TILES = 49                    # ceil(6250/128)
A_PC_PAD = A_TILES * 128        # 6272
N_A_PAD = N_CORES * A_PC_PAD    # 50176 padded author table rows
P_PC = N_PAPER // N_CORES       # 3125 papers per core
P_TILES = 25
P_PC_PAD = P_TILES * 128        # 3200
SPLIT = 32768                   # int16 index limit for dma_gather
GRP_C = 5                       # GCN: 10 groups x 5 tiles
GRP_W = 1                       # SAGE: groups of 1 tile (512B rows)

TRACE = False
LAST = {}

_CACHE = {}


def _pad_author_ids(v):
    """Map real author id -> padded table row (core-concat layout)."""
    return (v // A_PC) * A_PC_PAD + (v % A_PC)


def _wrap_idx(flat):
    """Lay out gather indices as [128, n/16] int16 (16-wrap, tiled x8)."""
    n = flat.shape[0]
    assert n % 16 == 0
    w = flat.reshape(n // 16, 16).T.astype(np.int16)  # [16, n/16]
    return np.tile(w, (8, 1))                          # [128, n/16]


def _prep_graph(src_pad, dst, w, n_dst_pc, n_tiles, grp,
                self_w=None):
    """Partition edges by destination shard/tile, split lo/hi by src row,
    group tiles for batched gathers, and build per-core idx + (dst_rel, w)
    tables.  Chunk counts are per-tile (shared across cores for SPMD).

    Returns (cfg, [(idx [128, TOT/16] i16, drel [128, TOTC] bf16,
                    w [128, TOTC] bf16) per core]).
    """
    E = src_pad.shape[0]
    core = dst // n_dst_pc
    ld = dst - core * n_dst_pc
    tile = ld >> 7
    drel = ld & 127
    hi = (src_pad >= SPLIT).astype(np.int64)

    key = (core * n_tiles + tile) * 2 + hi
    order = np.argsort(key, kind="stable")
    key_s = key[order]
    counts = np.bincount(key, minlength=N_CORES * n_tiles * 2)
    starts = np.concatenate([[0], np.cumsum(counts)[:-1]])
    rank = np.arange(E, dtype=np.int64) - starts[key_s]

    n_lo = counts[0::2].reshape(N_CORES, n_tiles)
    n_hi = counts[1::2].reshape(N_CORES, n_tiles)
    CLO = np.maximum(1, -(-n_lo.max(axis=0) // 128))  # [n_tiles]
    CHI = np.maximum(1, -(-n_hi.max(axis=0) // 128))
    CT = CLO + CHI

    gof = list(range(0, n_tiles, grp))
    gsz = [min(grp, n_tiles - g0) for g0 in gof]
    glo = [int(CLO[g0:g0 + s].sum()) for g0, s in zip(gof, gsz)]
    ghi = [int(CHI[g0:g0 + s].sum()) for g0, s in zip(gof, gsz)]
    gchunks = [a + b for a, b in zip(glo, ghi)]
    gbase = np.concatenate([[0], np.cumsum(np.array(gchunks) * 128)[:-1]])
    TOT = int(sum(gchunks)) * 128

    lo_off = np.zeros(n_tiles, np.int64)
    hi_off = np.zeros(n_tiles, np.int64)
    lo_cbase = np.zeros(n_tiles, np.int64)
    hi_cbase = np.zeros(n_tiles, np.int64)
    for gi, g0 in enumerate(gof):
        s = gsz[gi]
        cl = np.concatenate([[0], np.cumsum(CLO[g0:g0 + s])[:-1]])
        ch = np.concatenate([[0], np.cumsum(CHI[g0:g0 + s])[:-1]])
        for j in range(s):
            t = g0 + j
            lo_cbase[t] = cl[j]
            hi_cbase[t] = glo[gi] + ch[j]
            lo_off[t] = gbase[gi] + cl[j] * 128
            hi_off[t] = gbase[gi] + (glo[gi] + ch[j]) * 128

    if self_w is not None:
        CT = CT + 1  # trailing per-tile self chunk, filled by plain DMA
    chunk_off = np.concatenate([[0], np.cumsum(CT)[:-1]])
    TOTC = int(CT.sum())

    src_s = src_pad[order]
    drel_s = drel[order]
    w_s = w[order]
    core_s = core[order]
    tile_s = tile[order]
    hi_s = hi[order]
    idx_val = (src_s - hi_s * SPLIT).astype(np.int16)
    c_in_tile = np.where(hi_s == 0, rank >> 7, CLO[tile_s] + (rank >> 7))
    pos = np.where(hi_s == 0, lo_off[tile_s] + rank, hi_off[tile_s] + rank)
    colc = chunk_off[tile_s] + c_in_tile
    ep = rank & 127

    per_core = []
    for c in range(N_CORES):
        m = core_s == c
        flat = np.zeros(TOT, dtype=np.int16)
        flat[pos[m]] = idx_val[m]
        drel_t = np.zeros((128, TOTC), dtype=BF16)
        w_t = np.zeros((128, TOTC), dtype=BF16)
        drel_t[ep[m], colc[m]] = drel_s[m].astype(BF16)
        w_t[ep[m], colc[m]] = w_s[m].astype(BF16)
        if self_w is not None:
            selfcols = chunk_off + (CT - 1)  # [n_tiles]
            drel_t[:, selfcols] = np.arange(128, dtype=np.float32)[:, None]
            w_t[:, selfcols] = self_w[c].T.astype(BF16)
        per_core.append((_wrap_idx(flat), drel_t, w_t))

    cfg = dict(
        CLO=[int(x) for x in CLO], CHI=[int(x) for x in CHI],
        CT=[int(x) for x in CT],
        gof=gof, gsz=gsz, glo=glo, ghi=ghi, gchunks=gchunks,
        gbase=[int(x) for x in gbase],
        lo_cbase=[int(x) for x in lo_cbase],
        hi_cbase=[int(x) for x in hi_cbase],
        chunk_off=[int(x) for x in chunk_off],
        TOT=TOT, TOTC=TOTC, n_tiles=n_tiles,
        self_chunk=self_w is not None,
    )
    return cfg, per_core


def _prep(inputs):
    """Host preprocessing: per-core in_maps + static builder config."""
    f32 = np.float32
    x_author = np.asarray(inputs["x_author"], f32)
    paper_tokens = np.asarray(inputs["paper_tokens"])
    paper_feat = np.asarray(inputs["paper_feat"], f32)
    edge_collab = np.asarray(inputs["edge_collab"], np.int64)
    writes_src = np.asarray(inputs["writes_src"], np.int64)
    writes_dst = np.asarray(inputs["writes_dst"], np.int64)

    # -- padded bf16 author table, pre-interleaved as the low half of the
    # [xa | a1] SAGE message table (the device fills the a1 half after the
    # AllGather; idempotent across NEFF re-runs).
    xaa1 = np.zeros((N_A_PAD, 2 * D), dtype=BF16)
    rows = _pad_author_ids(np.arange(N_AUTHOR))
    xaa1[rows, :D] = x_author.astype(BF16)
    xa_pad = xaa1[:, :D]

    emb = np.asarray(inputs["embed_table"], f32).astype(BF16)

    # -- GCN (collab -> authors): norm = dinv[src]*dinv[dst]; self loop dinv^2
    src_c, dst_c = edge_collab[0], edge_collab[1]
    deg = np.bincount(dst_c, minlength=N_AUTHOR).astype(f32) + 1.0
    dinv = 1.0 / np.sqrt(deg)
    # self-loop messages come from a plain DMA of the core's own shard;
    # only their dinv^2 weights go through the (drel, w) tables
    sw = np.zeros((N_CORES, A_TILES, 128), f32)
    d2 = (dinv * dinv).reshape(N_CORES, A_PC)
    sw.reshape(N_CORES, A_TILES * 128)[:, :A_PC] = d2
    cfg_c, pc_c = _prep_graph(
        _pad_author_ids(src_c), dst_c,
        (dinv[src_c] * dinv[dst_c]).astype(f32), A_PC, A_TILES, grp=GRP_C,
        self_w=sw)

    # -- SAGE (writes -> papers): weight 1/max(cnt,1)
    cnt = np.bincount(writes_dst, minlength=N_PAPER).astype(f32)
    s_w = (1.0 / np.maximum(cnt, 1.0))[writes_dst].astype(f32)
    cfg_w, pc_w = _prep_graph(
        _pad_author_ids(writes_src), writes_dst, s_w, P_PC, P_TILES, grp=GRP_W)

    # -- weights (host-reshaped to device layouts, bf16)
    def brow(name, n):
        return np.asarray(inputs[name], f32).reshape(1, n).astype(BF16)

    Wp = np.asarray(inputs["Wp"], f32)  # [768, 768] [k, f]
    wp_sb = Wp.reshape(6, 128, D_BERT).transpose(1, 0, 2).reshape(128, 6 * D_BERT)
    Wr0 = np.asarray(inputs["sage_Wr0"], f32)  # [780, 128]
    wr0_sb = Wr0[:768].reshape(6, 128, 128).transpose(1, 0, 2).reshape(128, 6 * 128)
    wr0f = Wr0[768:780]  # [12, 128]
    bp_col = np.asarray(inputs["bp"], f32).reshape(6, 128).T  # [128, 6]

    iota = np.tile(np.arange(128, dtype=f32), (128, 1))  # [128, 128]
    ident = np.eye(128, dtype=f32)

    consts = dict(
        wp=wp_sb.astype(BF16),
        bpcol=bp_col.astype(BF16),
        w0=np.asarray(inputs["gcn_W0"], f32).astype(BF16),
        b0=brow("gcn_b0", D),
        wl0=np.asarray(inputs["sage_Wl0"], f32).astype(BF16),
        wr0=wr0_sb.astype(BF16),
        wr0f=wr0f.astype(BF16),
        sb0col=np.asarray(inputs["sage_b0"], f32).reshape(128, 1).astype(BF16),
        wl1=np.asarray(inputs["sage_Wl1"], f32).astype(BF16),
        wr1=np.asarray(inputs["sage_Wr1"], f32).astype(BF16),
        sb1col=np.asarray(inputs["sage_b1"], f32).reshape(128, 1).astype(BF16),
        linw=np.asarray(inputs["lin_W"], f32).astype(BF16),
        linb=brow("lin_b", OUT_D),
        ones=np.ones((1, 128), dtype=BF16),
        iota=iota.astype(BF16),
        ident=ident.astype(BF16),
    )

    in_maps = []
    for c in range(N_CORES):
        p0, p1 = c * P_PC, (c + 1) * P_PC
        cls = np.zeros(P_PC_PAD, dtype=np.int64)
        cls[:P_PC] = paper_tokens[p0:p1, 0]
        featT = np.zeros((FEAT, P_PC_PAD), dtype=BF16)
        featT[:, :P_PC] = paper_feat[p0:p1].T.astype(BF16)
        idx_c, drel_c, w_c = pc_c[c]
        idx_w, drel_w, w_w = pc_w[c]
        m = dict(
            emb=emb,
            clsidx=_wrap_idx(cls),
            featT=featT,
            xaa1=xaa1,
            xa_my=np.ascontiguousarray(
                xa_pad[c * A_PC_PAD:(c + 1) * A_PC_PAD]),
            idxc=idx_c, drelc=drel_c, wc=w_c,
            idxw=idx_w, drelw=drel_w, ww=w_w,
            **consts,
        )
        in_maps.append(m)

    return in_maps, (cfg_c, cfg_w)


def _cfg_key(cfg):
    return (tuple(cfg["CLO"]), tuple(cfg["CHI"]))


def _build(cfg_c, cfg_w):
    """Build the SPMD Bass program (shared by all 8 cores)."""
    import concourse.bacc as bacc
    import concourse.mybir as mybir
    from concourse.tile import TileContext

    dt = mybir.dt
    AF = mybir.ActivationFunctionType
    ALU = mybir.AluOpType

    nc = bacc.Bacc("TRN2", target_bir_lowering=False, debug=False,
                   num_devices=N_CORES, num_swdge_queues=4)

    # Round-robin over the 4 SWDGE queues so the SDMA drain of call N
    # overlaps the Q7 descriptor-gen of call N+1.  Tile tracks SWDGE
    # completion on 8 lanes assigned in execution order, assuming each
    # lane's DMAs finish FIFO; cross-queue completions reorder, so we (a)
    # chain all gathers with no-sync deps to pin execution order to
    # emission order and (b) rotate queues mod 4 — every lane (mod 8)
    # then maps to a fixed queue (mod 4), keeping each lane FIFO.
    qstate = {"q": 0, "hist": [], "barrier": False}

    def gather_rr(dst3, table, idx_ap, n, elem, single_packet=False,
                  transpose=False, elem_step=None):
        from concourse.instruction_name_ordered_set import (
            InstructionNameOrderedSet)
        q = qstate["q"]
        qstate["q"] = (q + 1) % 4
        bi = nc.gpsimd.dma_gather(dst3, table, idx_ap, n, n, elem,
                                  elem_step=elem_step,
                                  single_packet=single_packet,
                                  transpose=transpose, queue_num=q)
        hist = qstate["hist"]
        if hist:
            deps = InstructionNameOrderedSet()
            deps.add(hist[-1])
            bi.ins.add_nosync_dependencies_from(deps)
        if qstate["barrier"] and hist:
            # Phase boundary (collectives insert queue drains): wait for
            # ALL in-flight gathers before issuing the next one.
            qstate["barrier"] = False
            sdeps = InstructionNameOrderedSet()
            for nm in hist[-8:]:
                sdeps.add(nm)
            bi.ins.add_sync_dependencies_from(sdeps)
        elif len(hist) >= 8:
            # ≤1 DMA in flight per Tile DMASW lane (8 lanes): wait for the
            # lane predecessor's completion before issuing, so the shared
            # lane semaphore's 16 per-engine increments never interleave.
            sdeps = InstructionNameOrderedSet()
            sdeps.add(hist[-8])
            bi.ins.add_sync_dependencies_from(sdeps)
        hist.append(bi.ins.name)

    # ---- I/O declarations
    def din(name, shape, dtype=dt.bfloat16):
        return nc.dram_tensor(name, list(shape), dtype, kind="ExternalInput").ap()

    emb = din("emb", (VOCAB, D_BERT))
    clsidx = din("clsidx", (128, P_PC_PAD // 16), dt.int16)
    featT = din("featT", (FEAT, P_PC_PAD))
    xaa1 = din("xaa1", (N_A_PAD, 2 * D))
    xa_my = din("xa_my", (A_PC_PAD, D))
    wp = din("wp", (128, 6 * D_BERT))
    bpcol = din("bpcol", (128, 6))
    w0 = din("w0", (D, D))
    b0 = din("b0", (1, D))
    wl0 = din("wl0", (D, D))
    wr0 = din("wr0", (128, 6 * 128))
    wr0f = din("wr0f", (FEAT, D))
    sb0col = din("sb0col", (128, 1))
    wl1 = din("wl1", (D, D))
    wr1 = din("wr1", (D, D))
    sb1col = din("sb1col", (128, 1))
    linw = din("linw", (D, OUT_D))
    linb = din("linb", (1, OUT_D))
    ones = din("ones", (1, 128))
    iota = din("iota", (128, 128))
    ident = din("ident", (128, 128))
    idxc = din("idxc", (128, cfg_c["TOT"] // 16), dt.int16)
    drelc = din("drelc", (128, cfg_c["TOTC"]))
    wc = din("wc", (128, cfg_c["TOTC"]))
    idxw = din("idxw", (128, cfg_w["TOT"] // 16), dt.int16)
    drelw = din("drelw", (128, cfg_w["TOTC"]))
    ww = din("ww", (128, cfg_w["TOTC"]))
    out = nc.dram_tensor("out", [P_PC_PAD, OUT_D], dt.float32,
                         kind="ExternalOutput").ap()

    MSGW = max((max(cfg_c["gchunks"]) + GRP_C) * 128,
           max(cfg_w["gchunks"]) * 256)
    INDW = max(max(cfg_c["CT"]), max(cfg_w["CT"])) * 128

    with TileContext(nc) as tc:
        with (
            tc.tile_pool(name="const", bufs=1) as constp,
            tc.tile_pool(name="big", bufs=1) as bigp,
            tc.tile_pool(name="msg", bufs=4) as msgp,
            tc.tile_pool(name="ind", bufs=4) as indp,
            tc.tile_pool(name="cls", bufs=2) as clsp,
            tc.tile_pool(name="work", bufs=3) as workp,
            tc.tile_pool(name="psum", bufs=1, space="PSUM") as psump,
            tc.tile_pool(name="dram", bufs=1, space="DRAM") as dramp,
        ):
            # ---- constants to SBUF
            def load_const(ap_dram, name):
                t = constp.tile(list(ap_dram.shape), ap_dram.dtype, name=name)
                nc.sync.dma_start(out=t, in_=ap_dram)
                return t

            # GCN-critical tables first: the first gathers and their DVE
            # consumers gate the pipeline fill.
            idxc_sb = load_const(idxc, "idxc_sb")
            drelc_sb = load_const(drelc, "drelc_sb")
            wc_sb = load_const(wc, "wc_sb")
            iota_sb = load_const(iota, "iota_sb")
            w0_sb = load_const(w0, "w0_sb")
            b0_sb = load_const(b0, "b0_sb")
            ones_sb = load_const(ones, "ones_sb")
            idxw_sb = load_const(idxw, "idxw_sb")
            drelw_sb = load_const(drelw, "drelw_sb")
            ww_sb = load_const(ww, "ww_sb")
            wp_sb = load_const(wp, "wp_sb")
            bp_sb = load_const(bpcol, "bp_sb")
            wl0_sb = load_const(wl0, "wl0_sb")
            wr0_sb = load_const(wr0, "wr0_sb")
            wr0f_sb = load_const(wr0f, "wr0f_sb")
            sb0_sb = load_const(sb0col, "sb0_sb")
            wl1_sb = load_const(wl1, "wl1_sb")
            wr1_sb = load_const(wr1, "wr1_sb")
            sb1_sb = load_const(sb1col, "sb1_sb")
            linw_sb = load_const(linw, "linw_sb")
            linb_sb = load_const(linb, "linb_sb")
            ident_sb = load_const(ident, "ident_sb")
            clsidx_sb = load_const(clsidx, "clsidx_sb")
            featT_sb = load_const(featT, "featT_sb")

            s0staticT = bigp.tile([128, P_PC_PAD], dt.bfloat16, name="s0staticT")

            a1_shard = dramp.tile([A_PC_PAD, D], dt.bfloat16, name="a1_shard")
            R1 = 24 * 128  # author rows in the first collective half
            R2 = A_PC_PAD - R1
            a1_h1 = dramp.tile([N_CORES * R1, D], dt.bfloat16,
                               addr_space="Shared", name="a1_h1")
            a1_h2 = dramp.tile([N_CORES * R2, D], dt.bfloat16,
                               addr_space="Shared", name="a1_h2")
            # xa half is pre-interleaved by the host; only the a1 half is
            # written on device (idempotent across NEFF re-runs).
            xa_a1 = xaa1
            xa_a1_3d = xaa1[:, D:2 * D].rearrange("(c r) d -> c r d",
                                                  c=N_CORES)

            iota1 = iota_sb[:, :].rearrange("p (o e) -> p o e", o=1)

            # =========== shared message-pass emitter
            def message_pass(cfg, idx_sb, drel_sb, w_sb, table_lo, table_hi,
                             consume, pname, elem=128, agg_w=128,
                             self_table=None, gsel=None, table_step=None):
                CLO, CT = cfg["CLO"], cfg["CT"]
                nself = 1 if cfg.get("self_chunk") else 0
                for gi in (gsel if gsel is not None
                           else range(len(cfg["gof"]))):
                    g0 = cfg["gof"][gi]
                    s = cfg["gsz"][gi]
                    nch = cfg["gchunks"][gi]
                    glo_g = cfg["glo"][gi]
                    msg = msgp.tile([128, MSGW], dt.bfloat16, tag="msg",
                                    name=f"msg_{pname}")
                    m3 = msg[:, :(nch + nself * s) * elem].rearrange(
                        "p (c e) -> p c e", e=elem)
                    for j in range(s * nself):
                        t = g0 + j
                        nc.sync.dma_start(
                            out=m3[:, nch + j, :],
                            in_=self_table[t * 128:(t + 1) * 128, :])
                    base = cfg["gbase"][gi]
                    Nlo = glo_g * 128
                    Nhi = cfg["ghi"][gi] * 128

                    def gath(dst3, table, col0, nidx):
                        done = 0
                        while done < nidx:
                            n = min(8192, nidx - done)
                            gather_rr(
                                dst3[:, done // 128:(done + n) // 128, :],
                                table,
                                idx_sb[:, (col0 + done) // 16:
                                       (col0 + done + n) // 16],
                                n, elem, elem_step=table_step)
                            done += n

                    gath(m3[:, :glo_g, :], table_lo, base, Nlo)
                    gath(m3[:, glo_g:nch, :], table_hi, base + Nlo, Nhi)
                    for j in range(s):
                        t = g0 + j
                        C = CT[t]
                        co = cfg["chunk_off"][t]
                        ind = indp.tile([128, INDW], dt.bfloat16, tag="ind",
                                        name=f"ind_{pname}")
                        i3 = ind[:, :C * 128].rearrange("p (c e) -> p c e",
                                                        e=128)
                        db = drel_sb[:, co:co + C].rearrange(
                            "p (c o) -> p c o", o=1).to_broadcast([128, C, 128])
                        wb = w_sb[:, co:co + C].rearrange(
                            "p (c o) -> p c o", o=1).to_broadcast([128, C, 128])
                        ib = iota1.to_broadcast([128, C, 128])
                        nc.vector.tensor_tensor(i3, ib, db, ALU.is_equal)
                        nc.vector.tensor_tensor(i3, i3, wb, ALU.mult)
                        agg = psump.tile([128, 256], dt.float32, tag="agg",
                                         name=f"agg_{pname}", bufs=2)
                        lob = cfg["lo_cbase"][t]
                        hib = cfg["hi_cbase"][t]
                        for h in range(elem // 128):
                            for k in range(C):
                                if k >= C - nself:
                                    mi = nch + j
                                elif k < CLO[t]:
                                    mi = lob + k
                                else:
                                    mi = hib + (k - CLO[t])
                                nc.tensor.matmul(
                                    agg[:, h * 128:(h + 1) * 128],
                                    lhsT=m3[:, mi, h * 128:(h + 1) * 128],
                                    rhs=ind[:, k * 128:(k + 1) * 128],
                                    start=(k == 0), stop=(k == C - 1),
                                )
                        aggsb = workp.tile([128, 256], dt.bfloat16,
                                           tag="aggsb", name=f"aggsb_{pname}")
                        nc.scalar.activation(aggsb[:, :agg_w],
                                             agg[:, :agg_w], AF.Copy)
                        consume(t, aggsb)

            # =========== GCN layer 0 -> a1 shard, then AllGather
            def gcn_consume(t, aggsb):
                lp = psump.tile([128, 128], dt.float32, tag="layer",
                                name="lp_gcn", bufs=2)
                nc.tensor.matmul(lp, lhsT=aggsb[:, 0:128], rhs=w0_sb,
                                 start=True, stop=False)
                nc.tensor.matmul(lp, lhsT=ones_sb, rhs=b0_sb,
                                 start=False, stop=True)
                a1sb = workp.tile([128, 128], dt.bfloat16, tag="a1sb",
                                  name="a1sb")
                nc.scalar.activation(a1sb, lp, AF.Relu)
                nc.sync.dma_start(out=a1_shard[t * 128:(t + 1) * 128, :],
                                  in_=a1sb)

            with nc.named_scope("gcn"):
                message_pass(cfg_c, idxc_sb, drelc_sb, wc_sb,
                             xaa1[0:SPLIT, 0:D], xaa1[SPLIT:N_A_PAD, 0:D],
                             gcn_consume, "gcn", self_table=xa_my,
                             gsel=[0, 1, 2, 3, 4], table_step=2 * D)
            with nc.named_scope("ag1"):
                nc.gpsimd.collective_compute(
                    "AllGather", ALU.bypass,
                    replica_groups=[list(range(N_CORES))],
                    ins=[a1_shard[0:R1, :]], outs=[a1_h1.opt()],
                )
                nc.sync.dma_start(
                    out=xa_a1_3d[:, 0:R1, :],
                    in_=a1_h1[:, :].rearrange("(c r) d -> c r d", c=N_CORES))
            qstate["barrier"] = True
            with nc.named_scope("gcn2"):
                message_pass(cfg_c, idxc_sb, drelc_sb, wc_sb,
                             xaa1[0:SPLIT, 0:D], xaa1[SPLIT:N_A_PAD, 0:D],
                             gcn_consume, "gcn2", self_table=xa_my,
                             gsel=[5, 6, 7, 8, 9], table_step=2 * D)

            # =========== pooler + SAGE-0 static part, in 512-paper slabs
            qstate["barrier"] = True
            with nc.named_scope("pooler"):
                slabs = []
                off = 0
                while off < P_PC_PAD:
                    wdt = min(512, P_PC_PAD - off)
                    slabs.append((off, wdt))
                    off += wdt
                for soff, W in slabs:
                    clsT = clsp.tile([128, 6 * 512], dt.bfloat16, tag="cls",
                                     name="clsT")
                    c3 = clsT[:, :6 * W].rearrange("p (c e) -> p c e", e=W)
                    gather_rr(c3, emb,
                              clsidx_sb[:, soff // 16:(soff + W) // 16],
                              W, D_BERT, single_packet=True, transpose=True)
                    stat = psump.tile([128, 512], dt.float32, tag="stat",
                                      name="stat", bufs=1)
                    for ft in range(6):
                        ps = psump.tile([128, 512], dt.float32, tag="pool",
                                        name="ps_pool", bufs=2)
                        for cc in range(6):
                            nc.tensor.matmul(
                                ps[:, :W],
                                lhsT=wp_sb[:, cc * D_BERT + ft * 128:
                                           cc * D_BERT + ft * 128 + 128],
                                rhs=c3[:, cc, :],
                                start=(cc == 0), stop=(cc == 5),
                            )
                        pool_sb = workp.tile([128, 512], dt.bfloat16,
                                             tag="poolsb", name="pool_sb")
                        nc.scalar.activation(pool_sb[:, :W], ps[:, :W],
                                             AF.Tanh,
                                             bias=bp_sb[:, ft:ft + 1])
                        nc.tensor.matmul(
                            stat[:, :W],
                            lhsT=wr0_sb[:, ft * 128:(ft + 1) * 128],
                            rhs=pool_sb[:, :W],
                            start=(ft == 0), stop=False)
                    nc.tensor.matmul(stat[:, :W], lhsT=wr0f_sb,
                                     rhs=featT_sb[:, soff:soff + W],
                                     start=False, stop=True)
                    nc.scalar.activation(s0staticT[:, soff:soff + W],
                                         stat[:, :W], AF.Identity,
                                         bias=sb0_sb)

            with nc.named_scope("allgather"):
                nc.gpsimd.collective_compute(
                    "AllGather", ALU.bypass,
                    replica_groups=[list(range(N_CORES))],
                    ins=[a1_shard[R1:A_PC_PAD, :]], outs=[a1_h2.opt()],
                )
                nc.sync.dma_start(
                    out=xa_a1_3d[:, R1:A_PC_PAD, :],
                    in_=a1_h2[:, :].rearrange("(c r) d -> c r d", c=N_CORES))

            # =========== fused SAGE layers 0+1 + head -> out
            def sage_consume(t, aggsb):
                mean0 = aggsb[:, 0:128]
                mean1 = aggsb[:, 128:256]
                pp0 = psump.tile([128, 128], dt.float32, tag="layer",
                                 name="pp0", bufs=2)
                nc.tensor.matmul(pp0, lhsT=wl0_sb, rhs=mean0,
                                 start=True, stop=False)
                nc.tensor.matmul(pp0, lhsT=ident_sb,
                                 rhs=s0staticT[:, t * 128:(t + 1) * 128],
                                 start=False, stop=True)
                p1sb = workp.tile([128, 128], dt.bfloat16, tag="p1sb",
                                  name="p1sb")
                nc.scalar.activation(p1sb, pp0, AF.Relu)
                pp1 = psump.tile([128, 128], dt.float32, tag="layer",
                                 name="pp1", bufs=2)
                nc.tensor.matmul(pp1, lhsT=wl1_sb, rhs=mean1,
                                 start=True, stop=False)
                nc.tensor.matmul(pp1, lhsT=wr1_sb, rhs=p1sb,
                                 start=False, stop=True)
                p2sb = workp.tile([128, 128], dt.bfloat16, tag="p2sb",
                                  name="p2sb")
                nc.scalar.activation(p2sb, pp1, AF.Relu, bias=sb1_sb)
                hp = psump.tile([128, OUT_D], dt.float32, tag="head",
                                name="hp", bufs=1)
                nc.tensor.matmul(hp, lhsT=p2sb, rhs=linw_sb,
                                 start=True, stop=False)
                nc.tensor.matmul(hp, lhsT=ones_sb, rhs=linb_sb,
                                 start=False, stop=True)
                outsb = workp.tile([128, OUT_D], dt.float32, tag="outsb",
                                   name="outsb")
                nc.vector.tensor_copy(outsb, hp)
                nc.sync.dma_start(out=out[t * 128:(t + 1) * 128, :], in_=outsb)

            qstate["barrier"] = True
            with nc.named_scope("sage"):
                message_pass(cfg_w, idxw_sb, drelw_sb, ww_sb,
                             xa_a1[0:SPLIT, :], xa_a1[SPLIT:N_A_PAD, :],
                             sage_consume, "sg", elem=256, agg_w=256)

    nc.compile()
    return nc


def kernel(**inputs):
    from concourse import bass_utils

    in_maps, (cfg_c, cfg_w) = _prep(inputs)

    key = (_cfg_key(cfg_c), _cfg_key(cfg_w))
    if key in _CACHE:
        nc = _CACHE[key]
    else:
        nc = _build(cfg_c, cfg_w)
        _CACHE[key] = nc

    res = bass_utils.run_bass_kernel_spmd(
        nc, in_maps, core_ids=list(range(N_CORES)), trace=TRACE)
    LAST["exec_time_ns"] = res.exec_time_ns
    LAST["results"] = res

    pieces = [res.results[c]["out"][:P_PC] for c in range(N_CORES)]
    return np.concatenate(pieces, axis=0).astype(np.float32)

